# revision 4
# baseline (speedup 1.0000x reference)
"""BiRWKV (bidirectional RWKV attention) Trainium2 kernel.

kernel(**inputs) -> np.ndarray, full shapes:
  r, k, v: [4, 2048, 1024] f32; time_emb: [4, 128]; w, u: [1, 1, 1024];
  time_w_proj, time_u_proj: [1024, 128]; layer_time_scale: [1] -> [4, 2048, 1024]

Channel-parallel over 8 cores (Cc=128 ch each), batches merged along the
free dim ([128, 8192] per core, seg b = cols [2048b, ...)). fp16 I/O and
intermediates (2x DVE perf modes); per-batch scans (fp32 state, fp32 decay
tile); the work after the den scans is pipelined in two 2-batch halves so
the ACT division chain overlaps the num-side scans.

Math (double-scan identity):
  ek = exp(k);  ekd = E2*ek (tensor_scalar);  ekv = ek*v'';  ekdv = ekd*v''
  A = scanF(d, x); B = scanB(d, A); B[T-TL:] += dtail*A[T-1]
  num' = ekdv + B_n ; den' = ekd + B_d
  out = num' * exp(-ln(den' * (1 + e^-r)))     [= sig(r)*num/den, osc in v'']

Engines: ACT: exp(k), exp(-r), +1 (identity bias), ln, exp(-.)  [one
natural_log_exp table set + identity]; DVE: scans, ekd/ekv/ekdv, den'/num'
adds, final mul; Pool: decay broadcast tiles + tail fixes + den'' mul;
DMA split across the SP and Activation HWDGE rings.
"""
import os
import sys

sys.path.insert(0, "/opt/trn_rl_repo")

import numpy as np

import concourse.bacc as bacc
import concourse.mybir as mybir
from concourse import tile
from concourse.bass_utils import run_bass_kernel_spmd

# Prefer the table set that holds Exp AND Ln (and Identity) so the whole
# ACT chain (exp/ln/exp/identity) runs off one resident set instead of
# reloading on every Ln<->Exp alternation (~1.3us per reload).
from concourse import hw_specs as _hw_specs

_orig_get_act_tables = _hw_specs.get_activation_tables


def _nle_first_tables(arch):
    # Keep dict ORDER intact (position defines act_func_set_id); instead hide
    # Exp/Ln/Identity from every other set so the chooser is forced to the
    # one set that has all three.
    t = dict(_orig_get_act_tables(arch))
    key = "natural_log_exp_and_others"
    if key not in t:
        return t
    hide = {f for f in t[key]
            if str(f).split(".")[-1] in ("Exp", "Ln", "Identity")}
    out = {}
    for name, fns in t.items():
        out[name] = set(fns) if name == key else (set(fns) - hide)
    return out


bacc.get_activation_tables = _nle_first_tables

F32 = mybir.dt.float32
F16 = mybir.dt.float16
ALU = mybir.AluOpType
AF = mybir.ActivationFunctionType

P = 128
B = 4
T = 2048
W = B * T
TL = 128
H = W // 2            # half width (2 batches)
N_CORES = 8
LAST_EXEC_TIME_NS = None


def _build_kernel(nc, repeat=1):
    k_m = nc.dram_tensor("k", [P, W], F16, kind="ExternalInput").ap()
    v_m = nc.dram_tensor("v", [P, W], F16, kind="ExternalInput").ap()
    r_m = nc.dram_tensor("r", [P, W], F16, kind="ExternalInput").ap()
    dec = nc.dram_tensor("dec", [P, B], F32, kind="ExternalInput").ap()
    e2d = nc.dram_tensor("e2", [P, B], F32, kind="ExternalInput").ap()
    dd1 = nc.dram_tensor("dd1", [P, B], F32, kind="ExternalInput").ap()
    out = nc.dram_tensor("out", [P, W], F16, kind="ExternalOutput").ap()

    with tile.TileContext(nc) as tc:
        with tc.tile_pool(name="mp", bufs=1) as mp:
            t_a = mp.tile([P, W], F16, name="t_a")   # k -> ek
            t_b = mp.tile([P, W], F16, name="t_b")   # B_d
            t_c = mp.tile([P, W], F16, name="t_c")   # v'' -> A_n
            t_d = mp.tile([P, W], F16, name="t_d")   # ekv
            t_e = mp.tile([P, W], F16, name="t_e")   # B_n -> num' -> q
            t_f = mp.tile([P, W], F16, name="t_f")   # A_d -> den' -> den''
            t_g = mp.tile([P, W], F16, name="t_g")   # r -> enr -> enr1 -> rds
            t_h = mp.tile([P, H], F32, name="t_h")   # lden (per half)
            dms = [mp.tile([P, T], F32, name=f"dm{i}") for i in range(B)]
            dec_sb = mp.tile([P, B], F32, name="dec_sb")
            e2_sb = mp.tile([P, B], F32, name="e2_sb")
            dd1_sb = mp.tile([P, B], F32, name="dd1_sb")
            s0d = mp.tile([P, B], F32, name="s0d")
            s0n = mp.tile([P, B], F32, name="s0n")

            def seg(t, b):
                return t[:, b * T : (b + 1) * T]

            def half(t, h):
                return t[:, h * H : (h + 1) * H]

            def ends(t):
                # [P, B] strided view of each segment's last column
                return t[:, T - 1 :: T]

            def body():
                # --- DMA lead-in: k_b0 alone on the scalar ring so the den
                # scans can start ~2us after t0; everything else on sync ---
                nc.sync.dma_start(out=dec_sb[:], in_=dec)
                nc.sync.dma_start(out=e2_sb[:], in_=e2d)
                nc.sync.dma_start(out=dd1_sb[:], in_=dd1)
                nc.scalar.dma_start(out=seg(t_a, 0), in_=k_m[:, 0:T])
                nc.scalar.activation(seg(t_a, 0), seg(t_a, 0), AF.Exp)
                nc.sync.dma_start(out=seg(t_a, 1), in_=k_m[:, T : 2 * T])
                nc.scalar.activation(seg(t_a, 1), seg(t_a, 1), AF.Exp)
                nc.sync.dma_start(out=half(t_a, 1), in_=k_m[:, H:W])
                nc.sync.dma_start(out=t_c[:], in_=v_m)
                nc.sync.dma_start(out=t_g[:], in_=r_m)
                for b in (2, 3):
                    nc.scalar.activation(seg(t_a, b), seg(t_a, b), AF.Exp)

                # decay broadcast tiles (Pool)
                for b in range(B):
                    nc.gpsimd.tensor_copy(
                        out=dms[b][:],
                        in_=dec_sb[:, b : b + 1].broadcast_to([P, T]))

                for h in range(2):
                    nc.scalar.activation(half(t_g, h), half(t_g, h),
                                         AF.Exp, scale=-1.0)
                    nc.scalar.activation(half(t_g, h), half(t_g, h),
                                         AF.Identity, bias=1.0)

                # Pool: ekv early (num side), before den'' ops queue up
                for h in range(2):
                    nc.gpsimd.tensor_mul(half(t_d, h), half(t_a, h),
                                         half(t_c, h))            # ekv

                # den side per half: A_d scans, s0 seed, B_d scans, den' STT
                for h in range(2):
                    hb = (2 * h, 2 * h + 1)
                    for b in hb:
                        nc.vector.tensor_tensor_scan(
                            out=seg(t_f, b), data0=dms[b][:],
                            data1=seg(t_a, b),
                            initial=0.0, op0=ALU.mult, op1=ALU.add)   # A_d
                    nc.vector.tensor_mul(s0d[:, 2 * h : 2 * h + 2],
                                         ends(t_f)[:, 2 * h : 2 * h + 2],
                                         dd1_sb[:, 2 * h : 2 * h + 2])
                    for b in hb:
                        nc.vector.tensor_tensor_scan(
                            out=seg(t_b, b)[:, ::-1], data0=dms[b][:][:, ::-1],
                            data1=seg(t_f, b)[:, ::-1],
                            initial=s0d[:, b : b + 1],
                            op0=ALU.mult, op1=ALU.add)                # B_d
                    for b in hb:
                        nc.vector.scalar_tensor_tensor(
                            out=seg(t_f, b), in0=seg(t_a, b),
                            scalar=e2_sb[:, b : b + 1], in1=seg(t_b, b),
                            op0=ALU.mult, op1=ALU.add)            # den'
                    for b in hb:
                        nc.gpsimd.tensor_mul(seg(t_f, b), seg(t_f, b),
                                             seg(t_g, b))         # den'' (Pool)
                        nc.scalar.activation(t_h[:, (b % 2) * T : (b % 2 + 1) * T],
                                             seg(t_f, b), AF.Ln)
                        nc.scalar.activation(seg(t_g, b),
                                             t_h[:, (b % 2) * T : (b % 2 + 1) * T],
                                             AF.Exp, scale=-1.0)  # rds -> t_g

                # num side: A_n scans, s0 seed, B_n scans, num' STT, q, out
                for h in range(2):
                    hb = (2 * h, 2 * h + 1)
                    for b in hb:
                        nc.vector.tensor_tensor_scan(
                            out=seg(t_c, b), data0=dms[b][:],
                            data1=seg(t_d, b),
                            initial=0.0, op0=ALU.mult, op1=ALU.add)   # A_n
                    nc.vector.tensor_mul(s0n[:, 2 * h : 2 * h + 2],
                                         ends(t_c)[:, 2 * h : 2 * h + 2],
                                         dd1_sb[:, 2 * h : 2 * h + 2])
                    for b in hb:
                        nc.vector.tensor_tensor_scan(
                            out=seg(t_e, b)[:, ::-1], data0=dms[b][:][:, ::-1],
                            data1=seg(t_c, b)[:, ::-1],
                            initial=s0n[:, b : b + 1],
                            op0=ALU.mult, op1=ALU.add)            # B_n
                        nc.vector.scalar_tensor_tensor(
                            out=seg(t_e, b), in0=seg(t_d, b),
                            scalar=e2_sb[:, b : b + 1], in1=seg(t_e, b),
                            op0=ALU.mult, op1=ALU.add)            # num'
                    if h == 0:
                        nc.gpsimd.tensor_mul(half(t_e, h), half(t_e, h),
                                             half(t_g, h))        # q (Pool)
                        nc.scalar.dma_start(out=out[:, 0:H],
                                            in_=half(t_e, 0))
                    else:
                        # per-batch q+out so the final tail is one batch long
                        nc.gpsimd.tensor_mul(seg(t_e, 2), seg(t_e, 2),
                                             seg(t_g, 2))         # q (Pool)
                        nc.scalar.dma_start(out=out[:, 2 * T : 3 * T],
                                            in_=seg(t_e, 2))
                        nc.vector.tensor_mul(seg(t_e, 3), seg(t_e, 3),
                                             seg(t_g, 3))         # q
                        nc.sync.dma_start(out=out[:, 3 * T : 4 * T],
                                          in_=seg(t_e, 3))

            if repeat > 1:
                with tc.For_i(0, repeat):
                    body()
            else:
                body()
    return nc


def _host_prep(inputs, n_cores=N_CORES):
    r, k, v = inputs["r"], inputs["k"], inputs["v"]
    temb = np.asarray(inputs["time_emb"], dtype=np.float32)
    w = np.asarray(inputs["w"], dtype=np.float32)
    u = np.asarray(inputs["u"], dtype=np.float32)
    twp = np.asarray(inputs["time_w_proj"], dtype=np.float32)
    tup = np.asarray(inputs["time_u_proj"], dtype=np.float32)
    lts = np.asarray(inputs["layer_time_scale"], dtype=np.float32)

    Bf, Tf, C = k.shape
    Cc = C // n_cores

    w_cond = (w + (temb @ twp.T)[:, None, :] * lts)[:, 0, :].astype(np.float32)
    u_cond = (u + (temb @ tup.T)[:, None, :] * lts)[:, 0, :].astype(np.float32)
    tf = 1.0 / (1.0 + np.exp(-temb.sum(-1, dtype=np.float32)))
    decay = (np.exp(-np.exp(w_cond)) * (0.5 + 0.5 * tf)[:, None]).astype(np.float32)
    EU = np.exp(u_cond).astype(np.float32)
    osc = (0.8 + 0.2 * tf).astype(np.float32)

    d64 = decay.astype(np.float64)
    E2 = (EU / (1.0 - d64**2)).astype(np.float32)          # [B, C]
    dd1v = (d64 / (1.0 - d64**2)).astype(np.float32)       # [B, C]

    kT = k.transpose(2, 0, 1).reshape(C, Bf * Tf).astype(np.float16)
    vT = (v * osc[:, None, None]).transpose(2, 0, 1)
    vT = vT.reshape(C, Bf * Tf).astype(np.float16)
    rT = r.transpose(2, 0, 1).reshape(C, Bf * Tf).astype(np.float16)
    decT = decay.T.astype(np.float32)
    e2T = E2.T.astype(np.float32)
    dd1T = dd1v.T.astype(np.float32)

    in_maps = []
    for c0 in range(0, C, Cc):
        sl = slice(c0, c0 + Cc)
        in_maps.append({
            "k": np.ascontiguousarray(kT[sl]),
            "v": np.ascontiguousarray(vT[sl]),
            "r": np.ascontiguousarray(rT[sl]),
            "dec": np.ascontiguousarray(decT[sl]),
            "e2": np.ascontiguousarray(e2T[sl]),
            "dd1": np.ascontiguousarray(dd1T[sl]),
        })
    return in_maps, (Bf, Tf, C)


def kernel(**inputs) -> np.ndarray:
    global LAST_EXEC_TIME_NS
    in_maps, (Bf, Tf, C) = _host_prep(inputs)
    repeat = int(os.environ.get("KERNEL_REPEAT", "1"))
    nc = bacc.Bacc(num_devices=N_CORES)
    _build_kernel(nc, repeat=repeat)
    nc.compile()
    res = run_bass_kernel_spmd(nc, in_maps, core_ids=list(range(N_CORES)))
    LAST_EXEC_TIME_NS = res.exec_time_ns
    outT = np.concatenate([r_["out"] for r_ in res.results], axis=0)  # [C, B*T]
    o = outT.reshape(C, Bf, Tf).transpose(1, 2, 0)
    return np.ascontiguousarray(o).astype(inputs["r"].dtype, copy=False)


if __name__ == "__main__":
    Bq, Tq, Cq, TD = 4, 2048, 1024, 128
    rng = np.random.default_rng(0)
    demo = {
        "r": rng.standard_normal((Bq, Tq, Cq)).astype(np.float32),
        "k": rng.standard_normal((Bq, Tq, Cq)).astype(np.float32),
        "v": rng.standard_normal((Bq, Tq, Cq)).astype(np.float32),
        "time_emb": rng.standard_normal((Bq, TD)).astype(np.float32),
        "w": (0.1 * rng.standard_normal((1, 1, Cq))).astype(np.float32),
        "u": (0.1 * rng.standard_normal((1, 1, Cq))).astype(np.float32),
        "time_w_proj": (0.02 * rng.standard_normal((Cq, TD))).astype(np.float32),
        "time_u_proj": (0.02 * rng.standard_normal((Cq, TD))).astype(np.float32),
        "layer_time_scale": np.ones((1,), np.float32),
    }
    o = kernel(**demo)
    print(o.shape, o.dtype)


# revision 5
# speedup vs baseline: 1.0120x; 1.0120x over previous
"""BiRWKV (bidirectional RWKV attention) Trainium2 kernel.

kernel(**inputs) -> np.ndarray, full shapes:
  r, k, v: [4, 2048, 1024] f32; time_emb: [4, 128]; w, u: [1, 1, 1024];
  time_w_proj, time_u_proj: [1024, 128]; layer_time_scale: [1] -> [4, 2048, 1024]

Channel-parallel over 8 cores (Cc=128 ch each), batches merged along the
free dim ([128, 8192] per core, seg b = cols [2048b, ...)). fp16 I/O and
intermediates (2x DVE perf modes); per-batch scans (fp32 state, fp32 decay
tile); the work after the den scans is pipelined in two 2-batch halves so
the ACT division chain overlaps the num-side scans.

Math (double-scan identity):
  ek = exp(k);  ekd = E2*ek (tensor_scalar);  ekv = ek*v'';  ekdv = ekd*v''
  A = scanF(d, x); B = scanB(d, A); B[T-TL:] += dtail*A[T-1]
  num' = ekdv + B_n ; den' = ekd + B_d
  out = num' * exp(-ln(den' * (1 + e^-r)))     [= sig(r)*num/den, osc in v'']

Engines: ACT: exp(k), exp(-r), +1 (identity bias), ln, exp(-.)  [one
natural_log_exp table set + identity]; DVE: scans, ekd/ekv/ekdv, den'/num'
adds, final mul; Pool: decay broadcast tiles + tail fixes + den'' mul;
DMA split across the SP and Activation HWDGE rings.
"""
import os
import sys

sys.path.insert(0, "/opt/trn_rl_repo")

import numpy as np

import concourse.bacc as bacc
import concourse.mybir as mybir
from concourse import tile
from concourse.bass_utils import run_bass_kernel_spmd

# Prefer the table set that holds Exp AND Ln (and Identity) so the whole
# ACT chain (exp/ln/exp/identity) runs off one resident set instead of
# reloading on every Ln<->Exp alternation (~1.3us per reload).
from concourse import hw_specs as _hw_specs

_orig_get_act_tables = _hw_specs.get_activation_tables


def _nle_first_tables(arch):
    # Keep dict ORDER intact (position defines act_func_set_id); instead hide
    # Exp/Ln/Identity from every other set so the chooser is forced to the
    # one set that has all three.
    t = dict(_orig_get_act_tables(arch))
    key = "natural_log_exp_and_others"
    if key not in t:
        return t
    hide = {f for f in t[key]
            if str(f).split(".")[-1] in ("Exp", "Ln", "Identity")}
    out = {}
    for name, fns in t.items():
        out[name] = set(fns) if name == key else (set(fns) - hide)
    return out


bacc.get_activation_tables = _nle_first_tables

F32 = mybir.dt.float32
F16 = mybir.dt.float16
ALU = mybir.AluOpType
AF = mybir.ActivationFunctionType

P = 128
B = 4
T = 2048
W = B * T
TL = 128
H = W // 2            # half width (2 batches)
N_CORES = 8
LAST_EXEC_TIME_NS = None


def _build_kernel(nc, repeat=1):
    k_m = nc.dram_tensor("k", [P, W], F16, kind="ExternalInput").ap()
    v_m = nc.dram_tensor("v", [P, W], F16, kind="ExternalInput").ap()
    r_m = nc.dram_tensor("r", [P, W], F16, kind="ExternalInput").ap()
    dec = nc.dram_tensor("dec", [P, B], F32, kind="ExternalInput").ap()
    e2d = nc.dram_tensor("e2", [P, B], F32, kind="ExternalInput").ap()
    dd1 = nc.dram_tensor("dd1", [P, B], F32, kind="ExternalInput").ap()
    out = nc.dram_tensor("out", [P, W], F16, kind="ExternalOutput").ap()

    with tile.TileContext(nc) as tc:
        with tc.tile_pool(name="mp", bufs=1) as mp:
            t_a = mp.tile([P, W], F16, name="t_a")   # k -> ek
            t_b = mp.tile([P, W], F16, name="t_b")   # B_d
            t_c = mp.tile([P, W], F16, name="t_c")   # v'' -> A_n
            t_d = mp.tile([P, W], F16, name="t_d")   # ekv
            t_e = mp.tile([P, W], F16, name="t_e")   # B_n -> num' -> q
            t_f = mp.tile([P, W], F16, name="t_f")   # A_d -> den' -> den''
            t_g = mp.tile([P, W], F16, name="t_g")   # r -> enr -> enr1 -> rds
            t_h = mp.tile([P, H], F32, name="t_h")   # lden (per half)
            dms = [mp.tile([P, T], F32, name=f"dm{i}") for i in range(B)]
            dec_sb = mp.tile([P, B], F32, name="dec_sb")
            e2_sb = mp.tile([P, B], F32, name="e2_sb")
            dd1_sb = mp.tile([P, B], F32, name="dd1_sb")
            s0d = mp.tile([P, B], F32, name="s0d")
            s0n = mp.tile([P, B], F32, name="s0n")

            def seg(t, b):
                return t[:, b * T : (b + 1) * T]

            def half(t, h):
                return t[:, h * H : (h + 1) * H]

            def ends(t):
                # [P, B] strided view of each segment's last column
                return t[:, T - 1 :: T]

            def body():
                # --- DMA lead-in: k_b0 alone on the scalar ring so the den
                # scans can start ~2us after t0; everything else on sync ---
                nc.sync.dma_start(out=dec_sb[:], in_=dec)
                # batch 0 arrives in two 1024-wide pieces so exp/scan start
                # as early as possible (the first A_d scan is chained)
                hT = T // 2
                nc.scalar.dma_start(out=t_a[:, 0:hT], in_=k_m[:, 0:hT])
                nc.scalar.activation(t_a[:, 0:hT], t_a[:, 0:hT], AF.Exp)
                nc.sync.dma_start(out=t_a[:, hT:T], in_=k_m[:, hT:T])
                nc.scalar.activation(t_a[:, hT:T], t_a[:, hT:T], AF.Exp)
                nc.sync.dma_start(out=seg(t_a, 1), in_=k_m[:, T : 2 * T])
                nc.scalar.activation(seg(t_a, 1), seg(t_a, 1), AF.Exp)
                nc.sync.dma_start(out=half(t_a, 1), in_=k_m[:, H:W])
                # e2/dd1 are only read by the s0 muls and affine STTs (late)
                nc.sync.dma_start(out=e2_sb[:], in_=e2d)
                nc.sync.dma_start(out=dd1_sb[:], in_=dd1)
                nc.sync.dma_start(out=t_c[:], in_=v_m)
                nc.sync.dma_start(out=t_g[:], in_=r_m)
                for b in (2, 3):
                    nc.scalar.activation(seg(t_a, b), seg(t_a, b), AF.Exp)

                # decay broadcast tiles (Pool)
                for b in range(B):
                    nc.gpsimd.tensor_copy(
                        out=dms[b][:],
                        in_=dec_sb[:, b : b + 1].broadcast_to([P, T]))

                for h in range(2):
                    nc.scalar.activation(half(t_g, h), half(t_g, h),
                                         AF.Exp, scale=-1.0)
                    nc.scalar.activation(half(t_g, h), half(t_g, h),
                                         AF.Identity, bias=1.0)

                # Pool: ekv early (num side), before den'' ops queue up
                for h in range(2):
                    nc.gpsimd.tensor_mul(half(t_d, h), half(t_a, h),
                                         half(t_c, h))            # ekv

                # den side per half: A_d scans, s0 seed, B_d scans, den' STT
                for h in range(2):
                    hb = (2 * h, 2 * h + 1)
                    for b in hb:
                        if b == 0:
                            # batch 0 split in two chained pieces (early start)
                            nc.vector.tensor_tensor_scan(
                                out=t_f[:, 0:hT], data0=dms[0][:, 0:hT],
                                data1=t_a[:, 0:hT],
                                initial=0.0, op0=ALU.mult, op1=ALU.add)
                            nc.vector.tensor_tensor_scan(
                                out=t_f[:, hT:T], data0=dms[0][:, hT:T],
                                data1=t_a[:, hT:T],
                                initial=t_f[:, hT - 1 : hT],
                                op0=ALU.mult, op1=ALU.add)
                            continue
                        nc.vector.tensor_tensor_scan(
                            out=seg(t_f, b), data0=dms[b][:],
                            data1=seg(t_a, b),
                            initial=0.0, op0=ALU.mult, op1=ALU.add)   # A_d
                    nc.vector.tensor_mul(s0d[:, 2 * h : 2 * h + 2],
                                         ends(t_f)[:, 2 * h : 2 * h + 2],
                                         dd1_sb[:, 2 * h : 2 * h + 2])
                    for b in hb:
                        nc.vector.tensor_tensor_scan(
                            out=seg(t_b, b)[:, ::-1], data0=dms[b][:][:, ::-1],
                            data1=seg(t_f, b)[:, ::-1],
                            initial=s0d[:, b : b + 1],
                            op0=ALU.mult, op1=ALU.add)                # B_d
                    for b in hb:
                        nc.vector.scalar_tensor_tensor(
                            out=seg(t_f, b), in0=seg(t_a, b),
                            scalar=e2_sb[:, b : b + 1], in1=seg(t_b, b),
                            op0=ALU.mult, op1=ALU.add)            # den'
                    for b in hb:
                        nc.gpsimd.tensor_mul(seg(t_f, b), seg(t_f, b),
                                             seg(t_g, b))         # den'' (Pool)
                        nc.scalar.activation(t_h[:, (b % 2) * T : (b % 2 + 1) * T],
                                             seg(t_f, b), AF.Ln)
                        nc.scalar.activation(seg(t_g, b),
                                             t_h[:, (b % 2) * T : (b % 2 + 1) * T],
                                             AF.Exp, scale=-1.0)  # rds -> t_g

                # num side: A_n scans, s0 seed, B_n scans, num' STT, q, out
                for h in range(2):
                    hb = (2 * h, 2 * h + 1)
                    for b in hb:
                        nc.vector.tensor_tensor_scan(
                            out=seg(t_c, b), data0=dms[b][:],
                            data1=seg(t_d, b),
                            initial=0.0, op0=ALU.mult, op1=ALU.add)   # A_n
                    nc.vector.tensor_mul(s0n[:, 2 * h : 2 * h + 2],
                                         ends(t_c)[:, 2 * h : 2 * h + 2],
                                         dd1_sb[:, 2 * h : 2 * h + 2])
                    for b in hb:
                        nc.vector.tensor_tensor_scan(
                            out=seg(t_e, b)[:, ::-1], data0=dms[b][:][:, ::-1],
                            data1=seg(t_c, b)[:, ::-1],
                            initial=s0n[:, b : b + 1],
                            op0=ALU.mult, op1=ALU.add)            # B_n
                        nc.vector.scalar_tensor_tensor(
                            out=seg(t_e, b), in0=seg(t_d, b),
                            scalar=e2_sb[:, b : b + 1], in1=seg(t_e, b),
                            op0=ALU.mult, op1=ALU.add)            # num'
                    if h == 0:
                        nc.gpsimd.tensor_mul(half(t_e, h), half(t_e, h),
                                             half(t_g, h))        # q (Pool)
                        nc.scalar.dma_start(out=out[:, 0:H],
                                            in_=half(t_e, 0))
                    else:
                        # per-batch q+out so the final tail is one batch long
                        nc.gpsimd.tensor_mul(seg(t_e, 2), seg(t_e, 2),
                                             seg(t_g, 2))         # q (Pool)
                        nc.scalar.dma_start(out=out[:, 2 * T : 3 * T],
                                            in_=seg(t_e, 2))
                        nc.vector.tensor_mul(seg(t_e, 3), seg(t_e, 3),
                                             seg(t_g, 3))         # q
                        nc.sync.dma_start(out=out[:, 3 * T : 4 * T],
                                          in_=seg(t_e, 3))

            if repeat > 1:
                with tc.For_i(0, repeat):
                    body()
            else:
                body()
    return nc


def _host_prep(inputs, n_cores=N_CORES):
    r, k, v = inputs["r"], inputs["k"], inputs["v"]
    temb = np.asarray(inputs["time_emb"], dtype=np.float32)
    w = np.asarray(inputs["w"], dtype=np.float32)
    u = np.asarray(inputs["u"], dtype=np.float32)
    twp = np.asarray(inputs["time_w_proj"], dtype=np.float32)
    tup = np.asarray(inputs["time_u_proj"], dtype=np.float32)
    lts = np.asarray(inputs["layer_time_scale"], dtype=np.float32)

    Bf, Tf, C = k.shape
    Cc = C // n_cores

    w_cond = (w + (temb @ twp.T)[:, None, :] * lts)[:, 0, :].astype(np.float32)
    u_cond = (u + (temb @ tup.T)[:, None, :] * lts)[:, 0, :].astype(np.float32)
    tf = 1.0 / (1.0 + np.exp(-temb.sum(-1, dtype=np.float32)))
    decay = (np.exp(-np.exp(w_cond)) * (0.5 + 0.5 * tf)[:, None]).astype(np.float32)
    EU = np.exp(u_cond).astype(np.float32)
    osc = (0.8 + 0.2 * tf).astype(np.float32)

    d64 = decay.astype(np.float64)
    E2 = (EU / (1.0 - d64**2)).astype(np.float32)          # [B, C]
    dd1v = (d64 / (1.0 - d64**2)).astype(np.float32)       # [B, C]

    kT = k.transpose(2, 0, 1).reshape(C, Bf * Tf).astype(np.float16)
    vT = (v * osc[:, None, None]).transpose(2, 0, 1)
    vT = vT.reshape(C, Bf * Tf).astype(np.float16)
    rT = r.transpose(2, 0, 1).reshape(C, Bf * Tf).astype(np.float16)
    decT = decay.T.astype(np.float32)
    e2T = E2.T.astype(np.float32)
    dd1T = dd1v.T.astype(np.float32)

    in_maps = []
    for c0 in range(0, C, Cc):
        sl = slice(c0, c0 + Cc)
        in_maps.append({
            "k": np.ascontiguousarray(kT[sl]),
            "v": np.ascontiguousarray(vT[sl]),
            "r": np.ascontiguousarray(rT[sl]),
            "dec": np.ascontiguousarray(decT[sl]),
            "e2": np.ascontiguousarray(e2T[sl]),
            "dd1": np.ascontiguousarray(dd1T[sl]),
        })
    return in_maps, (Bf, Tf, C)


def kernel(**inputs) -> np.ndarray:
    global LAST_EXEC_TIME_NS
    in_maps, (Bf, Tf, C) = _host_prep(inputs)
    repeat = int(os.environ.get("KERNEL_REPEAT", "1"))
    nc = bacc.Bacc(num_devices=N_CORES)
    _build_kernel(nc, repeat=repeat)
    nc.compile()
    res = run_bass_kernel_spmd(nc, in_maps, core_ids=list(range(N_CORES)))
    LAST_EXEC_TIME_NS = res.exec_time_ns
    outT = np.concatenate([r_["out"] for r_ in res.results], axis=0)  # [C, B*T]
    o = outT.reshape(C, Bf, Tf).transpose(1, 2, 0)
    return np.ascontiguousarray(o).astype(inputs["r"].dtype, copy=False)


if __name__ == "__main__":
    Bq, Tq, Cq, TD = 4, 2048, 1024, 128
    rng = np.random.default_rng(0)
    demo = {
        "r": rng.standard_normal((Bq, Tq, Cq)).astype(np.float32),
        "k": rng.standard_normal((Bq, Tq, Cq)).astype(np.float32),
        "v": rng.standard_normal((Bq, Tq, Cq)).astype(np.float32),
        "time_emb": rng.standard_normal((Bq, TD)).astype(np.float32),
        "w": (0.1 * rng.standard_normal((1, 1, Cq))).astype(np.float32),
        "u": (0.1 * rng.standard_normal((1, 1, Cq))).astype(np.float32),
        "time_w_proj": (0.02 * rng.standard_normal((Cq, TD))).astype(np.float32),
        "time_u_proj": (0.02 * rng.standard_normal((Cq, TD))).astype(np.float32),
        "layer_time_scale": np.ones((1,), np.float32),
    }
    o = kernel(**demo)
    print(o.shape, o.dtype)


# revision 6
# speedup vs baseline: 1.0244x; 1.0122x over previous
"""BiRWKV (bidirectional RWKV attention) Trainium2 kernel.

kernel(**inputs) -> np.ndarray, full shapes:
  r, k, v: [4, 2048, 1024] f32; time_emb: [4, 128]; w, u: [1, 1, 1024];
  time_w_proj, time_u_proj: [1024, 128]; layer_time_scale: [1] -> [4, 2048, 1024]

Channel-parallel over 8 cores (Cc=128 ch each), batches merged along the
free dim ([128, 8192] per core, seg b = cols [2048b, ...)). fp16 I/O and
intermediates (2x DVE perf modes); per-batch scans (fp32 state, fp32 decay
tile); the work after the den scans is pipelined in two 2-batch halves so
the ACT division chain overlaps the num-side scans.

Math (double-scan identity):
  ek = exp(k);  ekd = E2*ek (tensor_scalar);  ekv = ek*v'';  ekdv = ekd*v''
  A = scanF(d, x); B = scanB(d, A); B[T-TL:] += dtail*A[T-1]
  num' = ekdv + B_n ; den' = ekd + B_d
  out = num' * exp(-ln(den' * (1 + e^-r)))     [= sig(r)*num/den, osc in v'']

Engines: ACT: exp(k), exp(-r), +1 (identity bias), ln, exp(-.)  [one
natural_log_exp table set + identity]; DVE: scans, ekd/ekv/ekdv, den'/num'
adds, final mul; Pool: decay broadcast tiles + tail fixes + den'' mul;
DMA split across the SP and Activation HWDGE rings.
"""
import os
import sys

sys.path.insert(0, "/opt/trn_rl_repo")

import numpy as np

import concourse.bacc as bacc
import concourse.mybir as mybir
from concourse import tile
from concourse.bass_utils import run_bass_kernel_spmd

# Prefer the table set that holds Exp AND Ln (and Identity) so the whole
# ACT chain (exp/ln/exp/identity) runs off one resident set instead of
# reloading on every Ln<->Exp alternation (~1.3us per reload).
from concourse import hw_specs as _hw_specs

_orig_get_act_tables = _hw_specs.get_activation_tables


def _nle_first_tables(arch):
    # Keep dict ORDER intact (position defines act_func_set_id); instead hide
    # Exp/Ln/Identity from every other set so the chooser is forced to the
    # one set that has all three.
    t = dict(_orig_get_act_tables(arch))
    key = "natural_log_exp_and_others"
    if key not in t:
        return t
    hide = {f for f in t[key]
            if str(f).split(".")[-1] in ("Exp", "Ln", "Identity")}
    out = {}
    for name, fns in t.items():
        out[name] = set(fns) if name == key else (set(fns) - hide)
    return out


bacc.get_activation_tables = _nle_first_tables

F32 = mybir.dt.float32
F16 = mybir.dt.float16
ALU = mybir.AluOpType
AF = mybir.ActivationFunctionType

P = 128
B = 4
T = 2048
W = B * T
TL = 128
H = W // 2            # half width (2 batches)
N_CORES = 8
LAST_EXEC_TIME_NS = None


def _build_kernel(nc, repeat=1):
    k_m = nc.dram_tensor("k", [P, W], F16, kind="ExternalInput").ap()
    v_m = nc.dram_tensor("v", [P, W], F16, kind="ExternalInput").ap()
    r_m = nc.dram_tensor("r", [P, W], F16, kind="ExternalInput").ap()
    dec = nc.dram_tensor("dec", [P, B], F32, kind="ExternalInput").ap()
    e2d = nc.dram_tensor("e2", [P, B], F32, kind="ExternalInput").ap()
    dd1 = nc.dram_tensor("dd1", [P, B], F32, kind="ExternalInput").ap()
    out = nc.dram_tensor("out", [P, W], F16, kind="ExternalOutput").ap()

    with tile.TileContext(nc) as tc:
        with tc.tile_pool(name="mp", bufs=1) as mp:
            t_a = mp.tile([P, W], F16, name="t_a")   # k -> ek
            t_b = mp.tile([P, W], F16, name="t_b")   # B_d
            t_c = mp.tile([P, W], F16, name="t_c")   # v'' -> A_n
            t_d = mp.tile([P, W], F16, name="t_d")   # ekv
            t_e = mp.tile([P, W], F16, name="t_e")   # B_n -> num' -> q
            t_f = mp.tile([P, W], F16, name="t_f")   # A_d -> den' -> den''
            t_g = mp.tile([P, W], F16, name="t_g")   # r -> enr -> enr1 -> rds
            t_h = mp.tile([P, H], F32, name="t_h")   # lden (per half)
            dms = [mp.tile([P, T], F32, name=f"dm{i}") for i in range(B)]
            dec_sb = mp.tile([P, B], F32, name="dec_sb")
            e2_sb = mp.tile([P, B], F32, name="e2_sb")
            dd1_sb = mp.tile([P, B], F32, name="dd1_sb")
            s0d = mp.tile([P, B], F32, name="s0d")
            s0n = mp.tile([P, B], F32, name="s0n")

            def seg(t, b):
                return t[:, b * T : (b + 1) * T]

            def half(t, h):
                return t[:, h * H : (h + 1) * H]

            def ends(t):
                # [P, B] strided view of each segment's last column
                return t[:, T - 1 :: T]

            def body():
                # --- DMA lead-in: k_b0 alone on the scalar ring so the den
                # scans can start ~2us after t0; everything else on sync ---
                nc.sync.dma_start(out=dec_sb[:], in_=dec)
                # batch 0 arrives in two 1024-wide pieces so exp/scan start
                # as early as possible (the first A_d scan is chained)
                hT = T // 2
                nc.scalar.dma_start(out=t_a[:, 0:hT], in_=k_m[:, 0:hT])
                nc.scalar.activation(t_a[:, 0:hT], t_a[:, 0:hT], AF.Exp)
                nc.sync.dma_start(out=t_a[:, hT:T], in_=k_m[:, hT:T])
                nc.scalar.activation(t_a[:, hT:T], t_a[:, hT:T], AF.Exp)
                nc.sync.dma_start(out=seg(t_a, 1), in_=k_m[:, T : 2 * T])
                nc.scalar.activation(seg(t_a, 1), seg(t_a, 1), AF.Exp)
                nc.sync.dma_start(out=half(t_a, 1), in_=k_m[:, H:W])
                # e2/dd1 are only read by the s0 muls and affine STTs (late)
                nc.sync.dma_start(out=e2_sb[:], in_=e2d)
                nc.sync.dma_start(out=dd1_sb[:], in_=dd1)
                nc.sync.dma_start(out=t_c[:], in_=v_m)
                nc.sync.dma_start(out=t_g[:], in_=r_m)
                for b in (2, 3):
                    nc.scalar.activation(seg(t_a, b), seg(t_a, b), AF.Exp)

                # decay broadcast tiles (Pool)
                for b in range(B):
                    nc.gpsimd.tensor_copy(
                        out=dms[b][:],
                        in_=dec_sb[:, b : b + 1].broadcast_to([P, T]))

                for h in range(2):
                    nc.scalar.activation(half(t_g, h), half(t_g, h),
                                         AF.Exp, scale=-1.0)
                    nc.scalar.activation(half(t_g, h), half(t_g, h),
                                         AF.Identity, bias=1.0)

                # Pool: ekv early (num side), before den'' ops queue up
                for h in range(2):
                    nc.gpsimd.tensor_mul(half(t_d, h), half(t_a, h),
                                         half(t_c, h))            # ekv

                # den side per half: A_d scans, s0 seed, B_d scans, den' STT
                for h in range(2):
                    hb = (2 * h, 2 * h + 1)
                    for b in hb:
                        if b == 0:
                            # batch 0 split in two chained pieces (early start)
                            nc.vector.tensor_tensor_scan(
                                out=t_f[:, 0:hT], data0=dms[0][:, 0:hT],
                                data1=t_a[:, 0:hT],
                                initial=0.0, op0=ALU.mult, op1=ALU.add)
                            nc.vector.tensor_tensor_scan(
                                out=t_f[:, hT:T], data0=dms[0][:, hT:T],
                                data1=t_a[:, hT:T],
                                initial=t_f[:, hT - 1 : hT],
                                op0=ALU.mult, op1=ALU.add)
                            continue
                        nc.vector.tensor_tensor_scan(
                            out=seg(t_f, b), data0=dms[b][:],
                            data1=seg(t_a, b),
                            initial=0.0, op0=ALU.mult, op1=ALU.add)   # A_d
                    nc.vector.tensor_mul(s0d[:, 2 * h : 2 * h + 2],
                                         ends(t_f)[:, 2 * h : 2 * h + 2],
                                         dd1_sb[:, 2 * h : 2 * h + 2])
                    for b in hb:
                        nc.vector.tensor_tensor_scan(
                            out=seg(t_b, b)[:, ::-1], data0=dms[b][:][:, ::-1],
                            data1=seg(t_f, b)[:, ::-1],
                            initial=s0d[:, b : b + 1],
                            op0=ALU.mult, op1=ALU.add)                # B_d
                    for b in hb:
                        nc.vector.scalar_tensor_tensor(
                            out=seg(t_f, b), in0=seg(t_a, b),
                            scalar=e2_sb[:, b : b + 1], in1=seg(t_b, b),
                            op0=ALU.mult, op1=ALU.add)            # den'
                    for b in hb:
                        nc.gpsimd.tensor_mul(seg(t_f, b), seg(t_f, b),
                                             seg(t_g, b))         # den'' (Pool)
                        nc.scalar.activation(t_h[:, (b % 2) * T : (b % 2 + 1) * T],
                                             seg(t_f, b), AF.Ln)
                        nc.scalar.activation(seg(t_g, b),
                                             t_h[:, (b % 2) * T : (b % 2 + 1) * T],
                                             AF.Exp, scale=-1.0)  # rds -> t_g

                # num side: A_n scans, s0 seed, B_n scans, num' STT, q, out
                for h in range(2):
                    hb = (2 * h, 2 * h + 1)
                    for b in hb:
                        nc.vector.tensor_tensor_scan(
                            out=seg(t_c, b), data0=dms[b][:],
                            data1=seg(t_d, b),
                            initial=0.0, op0=ALU.mult, op1=ALU.add)   # A_n
                    nc.vector.tensor_mul(s0n[:, 2 * h : 2 * h + 2],
                                         ends(t_c)[:, 2 * h : 2 * h + 2],
                                         dd1_sb[:, 2 * h : 2 * h + 2])
                    for b in hb:
                        nc.vector.tensor_tensor_scan(
                            out=seg(t_e, b)[:, ::-1], data0=dms[b][:][:, ::-1],
                            data1=seg(t_c, b)[:, ::-1],
                            initial=s0n[:, b : b + 1],
                            op0=ALU.mult, op1=ALU.add)            # B_n
                        nc.vector.scalar_tensor_tensor(
                            out=seg(t_e, b), in0=seg(t_d, b),
                            scalar=e2_sb[:, b : b + 1], in1=seg(t_e, b),
                            op0=ALU.mult, op1=ALU.add)            # num'
                    if h == 0:
                        nc.gpsimd.tensor_mul(half(t_e, h), half(t_e, h),
                                             half(t_g, h))        # q (Pool)
                        nc.scalar.dma_start(out=out[:, 0:H],
                                            in_=half(t_e, 0))
                    else:
                        # per-batch q+out so the final tail is one batch long
                        nc.gpsimd.tensor_mul(seg(t_e, 2), seg(t_e, 2),
                                             seg(t_g, 2))         # q (Pool)
                        nc.scalar.dma_start(out=out[:, 2 * T : 3 * T],
                                            in_=seg(t_e, 2))
                        # final batch in two pieces so the very last DVE op
                        # and output DMA are half-width
                        lo, mid, hi2 = 3 * T, 3 * T + T // 2, 4 * T
                        nc.vector.tensor_mul(t_e[:, lo:mid], t_e[:, lo:mid],
                                             t_g[:, lo:mid])      # q lo
                        nc.sync.dma_start(out=out[:, lo:mid],
                                          in_=t_e[:, lo:mid])
                        nc.vector.tensor_mul(t_e[:, mid:hi2], t_e[:, mid:hi2],
                                             t_g[:, mid:hi2])     # q hi
                        nc.sync.dma_start(out=out[:, mid:hi2],
                                          in_=t_e[:, mid:hi2])

            if repeat > 1:
                with tc.For_i(0, repeat):
                    body()
            else:
                body()
    return nc


def _host_prep(inputs, n_cores=N_CORES):
    r, k, v = inputs["r"], inputs["k"], inputs["v"]
    temb = np.asarray(inputs["time_emb"], dtype=np.float32)
    w = np.asarray(inputs["w"], dtype=np.float32)
    u = np.asarray(inputs["u"], dtype=np.float32)
    twp = np.asarray(inputs["time_w_proj"], dtype=np.float32)
    tup = np.asarray(inputs["time_u_proj"], dtype=np.float32)
    lts = np.asarray(inputs["layer_time_scale"], dtype=np.float32)

    Bf, Tf, C = k.shape
    Cc = C // n_cores

    w_cond = (w + (temb @ twp.T)[:, None, :] * lts)[:, 0, :].astype(np.float32)
    u_cond = (u + (temb @ tup.T)[:, None, :] * lts)[:, 0, :].astype(np.float32)
    tf = 1.0 / (1.0 + np.exp(-temb.sum(-1, dtype=np.float32)))
    decay = (np.exp(-np.exp(w_cond)) * (0.5 + 0.5 * tf)[:, None]).astype(np.float32)
    EU = np.exp(u_cond).astype(np.float32)
    osc = (0.8 + 0.2 * tf).astype(np.float32)

    d64 = decay.astype(np.float64)
    E2 = (EU / (1.0 - d64**2)).astype(np.float32)          # [B, C]
    dd1v = (d64 / (1.0 - d64**2)).astype(np.float32)       # [B, C]

    kT = k.transpose(2, 0, 1).reshape(C, Bf * Tf).astype(np.float16)
    vT = (v * osc[:, None, None]).transpose(2, 0, 1)
    vT = vT.reshape(C, Bf * Tf).astype(np.float16)
    rT = r.transpose(2, 0, 1).reshape(C, Bf * Tf).astype(np.float16)
    decT = decay.T.astype(np.float32)
    e2T = E2.T.astype(np.float32)
    dd1T = dd1v.T.astype(np.float32)

    in_maps = []
    for c0 in range(0, C, Cc):
        sl = slice(c0, c0 + Cc)
        in_maps.append({
            "k": np.ascontiguousarray(kT[sl]),
            "v": np.ascontiguousarray(vT[sl]),
            "r": np.ascontiguousarray(rT[sl]),
            "dec": np.ascontiguousarray(decT[sl]),
            "e2": np.ascontiguousarray(e2T[sl]),
            "dd1": np.ascontiguousarray(dd1T[sl]),
        })
    return in_maps, (Bf, Tf, C)


def kernel(**inputs) -> np.ndarray:
    global LAST_EXEC_TIME_NS
    in_maps, (Bf, Tf, C) = _host_prep(inputs)
    repeat = int(os.environ.get("KERNEL_REPEAT", "1"))
    nc = bacc.Bacc(num_devices=N_CORES)
    _build_kernel(nc, repeat=repeat)
    nc.compile()
    res = run_bass_kernel_spmd(nc, in_maps, core_ids=list(range(N_CORES)))
    LAST_EXEC_TIME_NS = res.exec_time_ns
    outT = np.concatenate([r_["out"] for r_ in res.results], axis=0)  # [C, B*T]
    o = outT.reshape(C, Bf, Tf).transpose(1, 2, 0)
    return np.ascontiguousarray(o).astype(inputs["r"].dtype, copy=False)


if __name__ == "__main__":
    Bq, Tq, Cq, TD = 4, 2048, 1024, 128
    rng = np.random.default_rng(0)
    demo = {
        "r": rng.standard_normal((Bq, Tq, Cq)).astype(np.float32),
        "k": rng.standard_normal((Bq, Tq, Cq)).astype(np.float32),
        "v": rng.standard_normal((Bq, Tq, Cq)).astype(np.float32),
        "time_emb": rng.standard_normal((Bq, TD)).astype(np.float32),
        "w": (0.1 * rng.standard_normal((1, 1, Cq))).astype(np.float32),
        "u": (0.1 * rng.standard_normal((1, 1, Cq))).astype(np.float32),
        "time_w_proj": (0.02 * rng.standard_normal((Cq, TD))).astype(np.float32),
        "time_u_proj": (0.02 * rng.standard_normal((Cq, TD))).astype(np.float32),
        "layer_time_scale": np.ones((1,), np.float32),
    }
    o = kernel(**demo)
    print(o.shape, o.dtype)
